# revision 42
# baseline (speedup 1.0000x reference)
"""Trainium2 kernel for nn_Encoder_68693706932594 (2-layer GCN encoder, GAE-style).

Math:
    deg = in-degree over all edges (self loops + hub edges included)
    dinv = deg^-1/2;  A_hat edges carry dinv[src]*dinv[dst]
    hidden1 = relu(A_hat @ x @ W1 + b1)
    mu      = A_hat @ hidden1 @ W2a + b2a
    logstd  = A_hat @ hidden1 @ W2b + b2b

Structure (v5 — contiguous message stream, no on-device gather):
  * A_hat(X W) == (A_hat X) W  -> aggregate raw (dinv[src]-scaled) features,
    then apply the dense [F,F] transform to the aggregated result.  mu and
    logstd share one aggregation, so two sparse passes total (one per launch,
    hidden1 round-trips through the host between launches).
  * The gather indices are host-known, so the host lays the messages out as a
    dense fp16 ELL stream in the exact order the device consumes:
    destination nodes are degree-sorted (self-loops count as ordinary edges;
    only hub-destination edges are excluded) and dealt round-robin to the 8
    cores; each core's 6250 nodes form 49 tiles of 128 lanes (tile ELL depth
    K[t] non-increasing) and slot layer s is a contiguous [128, w_s*96] fp16
    block covering the prefix of tiles with K[t] > s.
  * Accumulation: layers are combined in GROUPS of 8.  Within a group the
    layers are summed in fp16 into the group's first layer block (packed-2x
    DVE mode, flat contiguous APs), then each group total is added into a
    resident f32 accumulator (group 0 initializes it via tensor_copy).  This
    bounds the fp16 rounding staircase to the group depth while keeping most
    adds at the fast 16-bit rate.  All adds are split between DVE and GPSIMD
    by tile range (disjoint columns; separate split points for the fp16 and
    the f32-mixed work, balanced with measured rates).
  * Per tile: scalar-engine exact Copy quantizes acc to fp16 WITH the
    per-lane dinv_dst scale folded in (the scale commutes through the
    matmul); col 96 is a constant 1.0 bias channel (weight row 96 holds the
    bias).  PE transposes (batched 3 per PSUM tile), one PE matmul per tile
    against the combined [97,192] weight pair, exact scalar-engine copies
    (batched 2 tiles) into a partition-major fp16 staging buffer, stored in
    a few large chunks.  No activation tables anywhere (they are
    approximated on TRN2); layer 1's relu folds into the host-side
    inter-layer exchange.
  * The hub node (in-degree ~50k) is excluded and its rows patched on host.
"""

import numpy as np

import concourse.bacc as bacc
import concourse.mybir as mybir
import concourse.tile as tile
from concourse.bass_utils import run_bass_kernel_spmd
from concourse.masks import make_identity

P = 128          # partitions / tile lanes
F = 96           # feature dim
KC = 97          # matmul contraction: 96 feats + bias channel
N = 50000        # nodes
HUB = N - 1
NCORES = 8
NPC = N // NCORES                # 6250 dst nodes per core
NTILES = (NPC + P - 1) // P      # 49
TROWS = NTILES * P               # 6272
GRP = 10                         # layers per fp16 partial-sum group
F32 = mybir.dt.float32
F16 = mybir.dt.float16

_NC_CACHE = {}
LAST_EXEC_NS = None              # list of per-launch exec_time_ns when profiling


# --------------------------------------------------------------------------
# host-side graph preprocessing (graph-dependent only, done once)
# --------------------------------------------------------------------------

def _preprocess(edge_index):
    src = np.asarray(edge_index[0], dtype=np.int64)
    dst = np.asarray(edge_index[1], dtype=np.int64)

    deg = np.bincount(dst, minlength=N).astype(np.float32)
    dinv = np.where(
        deg > 0, 1.0 / np.sqrt(np.maximum(deg, 1.0)), 0.0
    ).astype(np.float32)

    hub_mask = dst == HUB
    hub_srcs = src[hub_mask]
    keep = ~hub_mask                 # self-loops are ordinary edges
    ks = src[keep]
    kd = dst[keep]

    cnt = np.bincount(kd, minlength=N)
    gorder = np.argsort(-cnt, kind="stable")
    orders = np.full((NCORES, TROWS), -1, dtype=np.int64)
    for c in range(NCORES):
        orders[c, :NPC] = gorder[c::NCORES]

    pos_in_core = np.zeros(N, dtype=np.int64)
    core_of = np.zeros(N, dtype=np.int64)
    for c in range(NCORES):
        pos_in_core[orders[c, :NPC]] = np.arange(NPC)
        core_of[orders[c, :NPC]] = c

    # unified (max-over-cores) ELL depth per tile; non-increasing by the sort
    km = np.zeros((NCORES, NTILES), dtype=np.int64)
    for c in range(NCORES):
        v = orders[c]
        cv = np.where(v >= 0, cnt[np.maximum(v, 0)], 0)
        km[c] = cv.reshape(NTILES, P).max(axis=1)
    K = km.max(axis=0)
    assert np.all(np.diff(K) <= 0)
    NL = int(K.sum())
    widths = [int((K > s).sum()) for s in range(int(K[0]))]
    layer_off = np.zeros(len(widths) + 1, dtype=np.int64)
    np.cumsum(widths, out=layer_off[1:])
    assert layer_off[-1] == NL

    # per-edge slot position -> gather index table [core, lane, row] -> node
    o = np.argsort(kd, kind="stable")
    s_src = ks[o]
    s_dst = kd[o]
    rp = np.zeros(N + 1, dtype=np.int64)
    np.cumsum(np.bincount(s_dst, minlength=N), out=rp[1:])
    r = np.arange(len(s_dst)) - rp[s_dst]
    pos = pos_in_core[s_dst]
    c_of = core_of[s_dst]
    t_of = pos // P
    lane = pos % P
    row = layer_off[r] + t_of
    nidx = np.full((NCORES, P, NL), N, dtype=np.int32)   # N = zero row
    nidx[c_of, lane, row] = s_src.astype(np.int32)

    # per-core per-lane dinv of destination nodes
    dinv_lane = np.zeros((NCORES, P, NTILES), dtype=np.float32)
    posr = np.arange(TROWS)
    for c in range(NCORES):
        v = orders[c]
        dv = np.where(v >= 0, dinv[np.maximum(v, 0)], 0.0).astype(np.float32)
        dinv_lane[c, posr % P, posr // P] = dv

    return {
        "dinv": dinv,
        "hub_srcs": hub_srcs,
        "orders": orders,
        "nidx": nidx,
        "dinv_lane": dinv_lane,
        "widths": widths,
        "layer_off": layer_off,
        "K": K,
        "NL": NL,
    }


# --------------------------------------------------------------------------
# device program
# --------------------------------------------------------------------------

# measured engine rates, elems/cycle @0.96GHz (flat contiguous APs)
R_DVE16 = 1.5      # fp16 += fp16
R_DVEMX = 0.47     # f32 += fp16 (and fp16 -> f32 copy), measured on-device
R_POOL = 0.5       # gpsimd, any dtype combo


def _build(widths, layer_off, K, NL):
    nc = bacc.Bacc("TRN2", target_bir_lowering=False, debug=False,
                   num_devices=NCORES)
    nbr = nc.dram_tensor("nbr", [P, NL * F], F16, kind="ExternalInput")
    wab = nc.dram_tensor("wab", [KC, 2 * F], F16, kind="ExternalInput")
    dinvl = nc.dram_tensor("dinvl", [P, NTILES], F32, kind="ExternalInput")
    outab = nc.dram_tensor("outab", [P, NTILES * 2 * F], F16,
                           kind="ExternalOutput")

    S = len(widths)
    # group boundaries at ELL-depth quantiles of the tiles, so every group
    # completion releases a similar number of tiles for their epilogues
    qs = sorted({int(K[q]) for q in (40, 32, 24, 16, 8)} | {S})
    gb = [0] + [b for b in qs if 0 < b <= S]
    if gb[-1] != S:
        gb.append(S)
    groups = [list(range(gb[i], gb[i + 1])) for i in range(len(gb) - 1)]

    # independent DVE/GPSIMD split points for the fp16 adds (T0f) and the
    # f32 mixed adds/copies (T0m), balancing measured rates
    def cost(T0f, T0m):
        dve = pool = 0.0
        for gi, g in enumerate(groups):
            for s in g[1:]:
                w = widths[s]
                dve += min(w, T0f) / R_DVE16
                pool += max(w - T0f, 0) / R_POOL
            if gi == 0:
                continue       # group-0 acc init runs on the scalar engine
            w = widths[g[0]]
            dve += min(w, T0m) / R_DVEMX
            pool += max(w - T0m, 0) / R_POOL
        # DVE also carries the PSUM->fp16 stage casts (~15us, 1 unit=100ns),
        # which this row-level model doesn't see: bias the split toward
        # GPSIMD accordingly
        return max(dve + 150.0, pool)
    T0f, T0m = min(((a, b) for a in range(8, NTILES + 1)
                    for b in range(8, NTILES + 1)),
                   key=lambda ab: cost(*ab))

    # nbr DMA pieces at layer boundaries: 1-layer pieces first (fast ramp),
    # finer pieces near the end (late groups release the deep tiles)
    pieces = []
    start = 0
    acc_rows = 0
    lim = 1
    for s, w in enumerate(widths):
        acc_rows += w
        if acc_rows >= lim or s == S - 1:
            end = int(layer_off[s + 1])
            pieces.append((start, end))
            start = end
            acc_rows = 0
            lim = 1 if s < 2 else (84 if s < 12 else 24)
    assert not widths or pieces[-1][1] == NL

    with tile.TileContext(nc) as tc:
        with (
            tc.tile_pool(name="const", bufs=1) as pc,
            tc.tile_pool(name="stream", bufs=1) as ps,
            tc.tile_pool(name="wk", bufs=1) as pk,
            tc.tile_pool(name="work", bufs=3) as pw,
            tc.tile_pool(name="pst", bufs=2, space="PSUM") as pst,
            tc.tile_pool(name="pso", bufs=3, space="PSUM") as pso,
        ):
            # tiny consts first (~0.2us), then the stream pieces
            dinv_sb = pc.tile([P, NTILES], F32)
            nc.sync.dma_start(dinv_sb[:], dinvl[:])
            wab0 = pc.tile([KC, 2 * F], F16)
            nc.sync.dma_start(wab0[:], wab[:])

            nbr_sb = ps.tile([P, NL * F], F16)
            for (a, b) in pieces:
                nc.sync.dma_start(nbr_sb[:, a * F:b * F], nbr[:, a * F:b * F])
            wab_sb = pc.tile([KC, 2 * F], F16)
            nc.scalar.copy(wab_sb[:], wab0[:])
            ident = pc.tile([P, P], F16)
            make_identity(nc, ident[:])

            # rotating transpose-input tiles with a persistent 1.0 bias
            # channel in col 96 (set once; the scaled copies write cols 0:96)
            a16s = [pk.tile([P, KC], F16, name=f"a16_{i}") for i in range(4)]
            for ai in a16s:
                nc.vector.memset(ai[:, F:KC], 1.0)

            acc = ps.tile([P, NTILES * F], F32)
            # staging buffer for both outputs; partition-major fp16
            stage = ps.tile([P, NTILES * 2 * F], F16)

            ai = [0]

            def epilogues(tlist):
                """Emit epilogues for a descending-consecutive tile list,
                in batches of 6 (transposes x3 per PSUM tile, matmul
                results x2 per PSUM tile)."""
                for b0 in range(0, len(tlist), 6):
                    batch = tlist[b0:b0 + 6]
                    pts = []
                    for j, t in enumerate(batch):
                        a16 = a16s[ai[0] % len(a16s)]
                        ai[0] += 1
                        # exact scaled copy: a16[:,:96] = dinv_dst*acc_tile
                        # (the scale commutes through the matmul; Copy is
                        # not table-based)
                        nc.scalar.activation(
                            a16[:, :F], acc[:, t * F:(t + 1) * F],
                            mybir.ActivationFunctionType.Copy,
                            scale=dinv_sb[:, t:t + 1],
                        )
                        j3 = j % 3
                        if j3 == 0:
                            n3 = min(3, len(batch) - j)
                            pt3 = pst.tile([P, 3 * P], F16, name="pt3",
                                           tag="pt3")
                            pts.append((pt3, n3))
                        nc.tensor.transpose(out=pt3[:KC, j3 * P:(j3 + 1) * P],
                                            in_=a16[:], identity=ident[:])
                    aggTs = []
                    for (pt3, n3) in pts:
                        aggT = pw.tile([P, 3 * P], F16, name="aggT",
                                       tag="aggT")
                        nc.scalar.copy(aggT[:KC, :n3 * P], pt3[:KC, :n3 * P])
                        aggTs.append(aggT)
                    for j0 in range(0, len(batch), 2):
                        n2 = min(2, len(batch) - j0)
                        pm2 = pso.tile([P, 4 * F], F32, name="pm2", tag="pm2")
                        for jj in range(n2):
                            j = j0 + jj
                            slot = n2 - 1 - jj     # ascending-t order
                            nc.tensor.matmul(
                                pm2[:, slot * 2 * F:(slot + 1) * 2 * F],
                                lhsT=aggTs[j // 3][:KC,
                                                   (j % 3) * P:(j % 3 + 1) * P],
                                rhs=wab_sb[:], start=True, stop=True)
                        lo_t = batch[j0 + n2 - 1]
                        # PSUM -> fp16 staging on DVE (idle once its share
                        # of the adds is done; scalar engine stays on the
                        # a16/aggT copies)
                        nc.vector.tensor_copy(
                            stage[:, lo_t * 2 * F:(lo_t + n2) * 2 * F],
                            pm2[:, :n2 * 2 * F])

            # interleave groups with the epilogues they release, so no
            # engine queue head-blocks on late data: after group g (layers
            # < B done) tiles with K <= B are final -- a contiguous
            # high-t suffix, emitted immediately and stored as one chunk
            K_arr = list(K)
            released = NTILES        # tiles >= released are already done
            for gi, g in enumerate(groups):
                o0 = int(layer_off[g[0]])
                # fp16 partial sums into the group's first layer block
                for s in g[1:]:
                    off = int(layer_off[s])
                    w = widths[s]
                    wd = min(w, T0f)
                    nc.vector.tensor_add(
                        nbr_sb[:, o0 * F:(o0 + wd) * F],
                        nbr_sb[:, o0 * F:(o0 + wd) * F],
                        nbr_sb[:, off * F:(off + wd) * F],
                    )
                    if w > T0f:
                        nc.gpsimd.tensor_add(
                            nbr_sb[:, (o0 + T0f) * F:(o0 + w) * F],
                            nbr_sb[:, (o0 + T0f) * F:(o0 + w) * F],
                            nbr_sb[:, (off + T0f) * F:(off + w) * F],
                        )
                # group total -> f32 accumulator (group 0 initializes)
                w = widths[g[0]]
                wd = min(w, T0m)
                if gi == 0:
                    # acc init as one exact scalar-engine Copy (fp16->f32),
                    # in Act's idle window: frees ~8.5us of DVE and ~8us of
                    # GPSIMD time and moves the first release earlier
                    nc.scalar.copy(acc[:, :w * F],
                                   nbr_sb[:, o0 * F:(o0 + w) * F])
                else:
                    nc.vector.tensor_add(
                        acc[:, :wd * F], acc[:, :wd * F],
                        nbr_sb[:, o0 * F:(o0 + wd) * F],
                    )
                    if w > T0m:
                        nc.gpsimd.tensor_add(
                            acc[:, T0m * F:w * F], acc[:, T0m * F:w * F],
                            nbr_sb[:, (o0 + T0m) * F:(o0 + w) * F],
                        )
                # release: all layers < B are folded into acc now
                B = g[-1] + 1
                tB = released
                while tB > 0 and K_arr[tB - 1] <= B:
                    tB -= 1
                if gi == len(groups) - 1:
                    tB = 0
                if tB < released:
                    epilogues(list(range(released - 1, tB - 1, -1)))
                    nc.sync.dma_start(
                        outab[:, tB * 2 * F:released * 2 * F],
                        stage[:, tB * 2 * F:released * 2 * F])
                    released = tB

    nc.compile()
    return nc


# --------------------------------------------------------------------------
# kernel entry point
# --------------------------------------------------------------------------

def kernel(x, W1, b1, W2a, b2a, W2b, b2b, edge_index, _profile=False):
    global LAST_EXEC_NS
    x = np.ascontiguousarray(np.asarray(x, dtype=np.float32))
    W1 = np.asarray(W1, dtype=np.float32)
    b1 = np.asarray(b1, dtype=np.float32)
    W2a = np.asarray(W2a, dtype=np.float32)
    b2a = np.asarray(b2a, dtype=np.float32)
    W2b = np.asarray(W2b, dtype=np.float32)
    b2b = np.asarray(b2b, dtype=np.float32)
    edge_index = np.asarray(edge_index)

    pp = _preprocess(edge_index)
    dinv = pp["dinv"]
    orders = pp["orders"]
    NL = pp["NL"]

    key = (NL, tuple(pp["widths"]))
    if key not in _NC_CACHE:
        _NC_CACHE.clear()
        _NC_CACHE[key] = _build(pp["widths"], pp["layer_off"], pp["K"], NL)
    nc = _NC_CACHE[key]

    def pad_wab(w_a, b_a, w_b, b_b):
        wp = np.zeros((KC, 2 * F), dtype=np.float32)
        wp[:F, :F] = w_a
        wp[F, :F] = b_a    # bias channel (paired with const 1.0 in a16 col 96)
        wp[:F, F:] = w_b
        wp[F, F:] = b_b
        return wp.astype(np.float16)

    exec_ns = []

    def build_streams(g32):
        """g32: [N, F] f32 dinv[src]-scaled features -> per-core fp16 stream."""
        g16pad = np.zeros((N + 1, F), dtype=np.float16)
        g16pad[:N] = g32.astype(np.float16)
        nbr_all = g16pad[pp["nidx"]]                   # [8, 128, NL, 96]
        return nbr_all.reshape(NCORES, P, NL * F)

    def launch(nbr_all, w_a, b_a, w_b, b_b):
        wab_p = pad_wab(w_a, b_a, w_b, b_b)
        in_maps = [
            {
                "nbr": nbr_all[c],
                "wab": wab_p,
                "dinvl": pp["dinv_lane"][c],
            }
            for c in range(NCORES)
        ]
        res = run_bass_kernel_spmd(nc, in_maps, core_ids=list(range(NCORES)),
                                   trace=bool(_profile))
        exec_ns.append(res.exec_time_ns)
        return res.results

    def assemble(res, half):
        full = np.zeros((N, F), dtype=np.float32)
        for c in range(NCORES):
            arr = res[c]["outab"].reshape(P, NTILES, 2 * F)
            part = arr[:, :, half * F:(half + 1) * F].transpose(1, 0, 2)
            full[orders[c, :NPC]] = part.reshape(TROWS, F)[:NPC]
        return full

    # ---- launch 1: hidden1 = relu(dinv_dst * (A x) W1 + b1) ----
    # (the linear part runs on device; relu folds into the host-side
    # inter-layer exchange)
    g_x = dinv[:, None] * x
    res1 = launch(build_streams(g_x), W1, b1, W1, b1)
    hidden1 = np.maximum(assemble(res1, 0), 0.0)
    s1 = g_x[pp["hub_srcs"]].sum(axis=0, dtype=np.float32)
    hidden1[HUB] = np.maximum((dinv[HUB] * s1) @ W1 + b1, 0.0)

    # ---- launch 2: mu / logstd from shared aggregation of hidden1 ----
    g_h = dinv[:, None] * hidden1
    res2 = launch(build_streams(g_h), W2a, b2a, W2b, b2b)
    mu = assemble(res2, 0)
    logstd = assemble(res2, 1)
    s2 = g_h[pp["hub_srcs"]].sum(axis=0, dtype=np.float32)
    mu[HUB] = (dinv[HUB] * s2) @ W2a + b2a
    logstd[HUB] = (dinv[HUB] * s2) @ W2b + b2b

    LAST_EXEC_NS = exec_ns
    return mu, logstd


# revision 43
# speedup vs baseline: 1.0768x; 1.0768x over previous
"""Trainium2 kernel for nn_Encoder_68693706932594 (2-layer GCN encoder, GAE-style).

Math:
    deg = in-degree over all edges (self loops + hub edges included)
    dinv = deg^-1/2;  A_hat edges carry dinv[src]*dinv[dst]
    hidden1 = relu(A_hat @ x @ W1 + b1)
    mu      = A_hat @ hidden1 @ W2a + b2a
    logstd  = A_hat @ hidden1 @ W2b + b2b

Structure (v5 — contiguous message stream, no on-device gather):
  * A_hat(X W) == (A_hat X) W  -> aggregate raw (dinv[src]-scaled) features,
    then apply the dense [F,F] transform to the aggregated result.  mu and
    logstd share one aggregation, so two sparse passes total (one per launch,
    hidden1 round-trips through the host between launches).
  * The gather indices are host-known, so the host lays the messages out as a
    dense fp16 ELL stream in the exact order the device consumes:
    destination nodes are degree-sorted (self-loops count as ordinary edges;
    only hub-destination edges are excluded) and dealt round-robin to the 8
    cores; each core's 6250 nodes form 49 tiles of 128 lanes (tile ELL depth
    K[t] non-increasing) and slot layer s is a contiguous [128, w_s*96] fp16
    block covering the prefix of tiles with K[t] > s.
  * Accumulation: layers are combined in GROUPS of 8.  Within a group the
    layers are summed in fp16 into the group's first layer block (packed-2x
    DVE mode, flat contiguous APs), then each group total is added into a
    resident f32 accumulator (group 0 initializes it via tensor_copy).  This
    bounds the fp16 rounding staircase to the group depth while keeping most
    adds at the fast 16-bit rate.  All adds are split between DVE and GPSIMD
    by tile range (disjoint columns; separate split points for the fp16 and
    the f32-mixed work, balanced with measured rates).
  * Per tile: scalar-engine exact Copy quantizes acc to fp16 WITH the
    per-lane dinv_dst scale folded in (the scale commutes through the
    matmul); col 96 is a constant 1.0 bias channel (weight row 96 holds the
    bias).  PE transposes (batched 3 per PSUM tile), one PE matmul per tile
    against the combined [97,192] weight pair, exact scalar-engine copies
    (batched 2 tiles) into a partition-major fp16 staging buffer, stored in
    a few large chunks.  No activation tables anywhere (they are
    approximated on TRN2); layer 1's relu folds into the host-side
    inter-layer exchange.
  * The hub node (in-degree ~50k) is excluded and its rows patched on host.
"""

import numpy as np

import concourse.bacc as bacc
import concourse.mybir as mybir
import concourse.tile as tile
from concourse.bass_utils import run_bass_kernel_spmd
from concourse.masks import make_identity

P = 128          # partitions / tile lanes
F = 96           # feature dim
KC = 97          # matmul contraction: 96 feats + bias channel
N = 50000        # nodes
HUB = N - 1
NCORES = 8
NPC = N // NCORES                # 6250 dst nodes per core
NTILES = (NPC + P - 1) // P      # 49
TROWS = NTILES * P               # 6272
GRP = 10                         # layers per fp16 partial-sum group
F32 = mybir.dt.float32
F16 = mybir.dt.float16

_NC_CACHE = {}
LAST_EXEC_NS = None              # list of per-launch exec_time_ns when profiling


# --------------------------------------------------------------------------
# host-side graph preprocessing (graph-dependent only, done once)
# --------------------------------------------------------------------------

def _preprocess(edge_index):
    src = np.asarray(edge_index[0], dtype=np.int64)
    dst = np.asarray(edge_index[1], dtype=np.int64)

    deg = np.bincount(dst, minlength=N).astype(np.float32)
    dinv = np.where(
        deg > 0, 1.0 / np.sqrt(np.maximum(deg, 1.0)), 0.0
    ).astype(np.float32)

    hub_mask = dst == HUB
    hub_srcs = src[hub_mask]
    keep = ~hub_mask                 # self-loops are ordinary edges
    ks = src[keep]
    kd = dst[keep]

    cnt = np.bincount(kd, minlength=N)
    gorder = np.argsort(-cnt, kind="stable")
    orders = np.full((NCORES, TROWS), -1, dtype=np.int64)
    for c in range(NCORES):
        orders[c, :NPC] = gorder[c::NCORES]

    pos_in_core = np.zeros(N, dtype=np.int64)
    core_of = np.zeros(N, dtype=np.int64)
    for c in range(NCORES):
        pos_in_core[orders[c, :NPC]] = np.arange(NPC)
        core_of[orders[c, :NPC]] = c

    # unified (max-over-cores) ELL depth per tile; non-increasing by the sort
    km = np.zeros((NCORES, NTILES), dtype=np.int64)
    for c in range(NCORES):
        v = orders[c]
        cv = np.where(v >= 0, cnt[np.maximum(v, 0)], 0)
        km[c] = cv.reshape(NTILES, P).max(axis=1)
    K = km.max(axis=0)
    assert np.all(np.diff(K) <= 0)
    NL = int(K.sum())
    widths = [int((K > s).sum()) for s in range(int(K[0]))]
    layer_off = np.zeros(len(widths) + 1, dtype=np.int64)
    np.cumsum(widths, out=layer_off[1:])
    assert layer_off[-1] == NL

    # per-edge slot position -> gather index table [core, lane, row] -> node
    o = np.argsort(kd, kind="stable")
    s_src = ks[o]
    s_dst = kd[o]
    rp = np.zeros(N + 1, dtype=np.int64)
    np.cumsum(np.bincount(s_dst, minlength=N), out=rp[1:])
    r = np.arange(len(s_dst)) - rp[s_dst]
    pos = pos_in_core[s_dst]
    c_of = core_of[s_dst]
    t_of = pos // P
    lane = pos % P
    row = layer_off[r] + t_of
    nidx = np.full((NCORES, P, NL), N, dtype=np.int32)   # N = zero row
    nidx[c_of, lane, row] = s_src.astype(np.int32)

    # per-core per-lane dinv of destination nodes
    dinv_lane = np.zeros((NCORES, P, NTILES), dtype=np.float32)
    posr = np.arange(TROWS)
    for c in range(NCORES):
        v = orders[c]
        dv = np.where(v >= 0, dinv[np.maximum(v, 0)], 0.0).astype(np.float32)
        dinv_lane[c, posr % P, posr // P] = dv

    return {
        "dinv": dinv,
        "hub_srcs": hub_srcs,
        "orders": orders,
        "nidx": nidx,
        "dinv_lane": dinv_lane,
        "widths": widths,
        "layer_off": layer_off,
        "K": K,
        "NL": NL,
    }


# --------------------------------------------------------------------------
# device program
# --------------------------------------------------------------------------

# measured engine rates, elems/cycle @0.96GHz (flat contiguous APs)
R_DVE16 = 1.5      # fp16 += fp16
R_DVEMX = 0.47     # f32 += fp16 (and fp16 -> f32 copy), measured on-device
R_POOL = 0.5       # gpsimd, any dtype combo


def _build(widths, layer_off, K, NL):
    nc = bacc.Bacc("TRN2", target_bir_lowering=False, debug=False,
                   num_devices=NCORES)
    nbr = nc.dram_tensor("nbr", [P, NL * F], F16, kind="ExternalInput")
    wab = nc.dram_tensor("wab", [KC, 2 * F], F16, kind="ExternalInput")
    dinvl = nc.dram_tensor("dinvl", [P, NTILES], F32, kind="ExternalInput")
    outab = nc.dram_tensor("outab", [P, NTILES * 2 * F], F16,
                           kind="ExternalOutput")

    S = len(widths)
    # group boundaries at ELL-depth quantiles of the tiles, so every group
    # completion releases a similar number of tiles for their epilogues
    qs = sorted({int(K[q]) for q in (40, 32, 24, 16, 8)} | {S})
    gb = [0] + [b for b in qs if 0 < b <= S]
    if gb[-1] != S:
        gb.append(S)
    groups = [list(range(gb[i], gb[i + 1])) for i in range(len(gb) - 1)]

    # independent DVE/GPSIMD split points for the fp16 adds (T0f) and the
    # f32 mixed adds/copies (T0m), balancing measured rates
    def cost(T0f, T0m):
        dve = pool = 0.0
        for gi, g in enumerate(groups):
            for s in g[1:]:
                w = widths[s]
                dve += min(w, T0f) / R_DVE16
                pool += max(w - T0f, 0) / R_POOL
            if gi == 0:
                continue       # group-0 acc init runs on the scalar engine
            w = widths[g[0]]
            dve += min(w, T0m) / R_DVEMX
            pool += max(w - T0m, 0) / R_POOL
        return max(dve, pool)
    T0f, T0m = min(((a, b) for a in range(8, NTILES + 1)
                    for b in range(8, NTILES + 1)),
                   key=lambda ab: cost(*ab))

    # nbr DMA pieces at layer boundaries: 1-layer pieces first (fast ramp),
    # finer pieces near the end (late groups release the deep tiles)
    pieces = []
    start = 0
    acc_rows = 0
    lim = 1
    for s, w in enumerate(widths):
        acc_rows += w
        if acc_rows >= lim or s == S - 1:
            end = int(layer_off[s + 1])
            pieces.append((start, end))
            start = end
            acc_rows = 0
            lim = 1 if s < 2 else (84 if s < 12 else 24)
    assert not widths or pieces[-1][1] == NL

    with tile.TileContext(nc) as tc:
        with (
            tc.tile_pool(name="const", bufs=1) as pc,
            tc.tile_pool(name="stream", bufs=1) as ps,
            tc.tile_pool(name="wk", bufs=1) as pk,
            tc.tile_pool(name="work", bufs=3) as pw,
            tc.tile_pool(name="pst", bufs=2, space="PSUM") as pst,
            tc.tile_pool(name="pso", bufs=3, space="PSUM") as pso,
        ):
            # tiny consts first (~0.2us), then the stream pieces
            dinv_sb = pc.tile([P, NTILES], F32)
            nc.sync.dma_start(dinv_sb[:], dinvl[:])
            wab0 = pc.tile([KC, 2 * F], F16)
            nc.sync.dma_start(wab0[:], wab[:])

            nbr_sb = ps.tile([P, NL * F], F16)
            for (a, b) in pieces:
                nc.sync.dma_start(nbr_sb[:, a * F:b * F], nbr[:, a * F:b * F])
            wab_sb = pc.tile([KC, 2 * F], F16)
            nc.scalar.copy(wab_sb[:], wab0[:])
            ident = pc.tile([P, P], F16)
            make_identity(nc, ident[:])

            # rotating transpose-input tiles with a persistent 1.0 bias
            # channel in col 96 (set once; the scaled copies write cols 0:96)
            a16s = [pk.tile([P, KC], F16, name=f"a16_{i}") for i in range(4)]
            for ai in a16s:
                nc.vector.memset(ai[:, F:KC], 1.0)

            acc = ps.tile([P, NTILES * F], F32)
            # staging buffer for both outputs; partition-major fp16
            stage = ps.tile([P, NTILES * 2 * F], F16)

            ai = [0]

            def epilogues(tlist):
                """Emit epilogues for a descending-consecutive tile list,
                in batches of 6 (transposes x3 per PSUM tile, matmul
                results x2 per PSUM tile)."""
                for b0 in range(0, len(tlist), 6):
                    batch = tlist[b0:b0 + 6]
                    pts = []
                    for j, t in enumerate(batch):
                        a16 = a16s[ai[0] % len(a16s)]
                        ai[0] += 1
                        # exact scaled copy: a16[:,:96] = dinv_dst*acc_tile
                        # (the scale commutes through the matmul; Copy is
                        # not table-based)
                        nc.scalar.activation(
                            a16[:, :F], acc[:, t * F:(t + 1) * F],
                            mybir.ActivationFunctionType.Copy,
                            scale=dinv_sb[:, t:t + 1],
                        )
                        j3 = j % 3
                        if j3 == 0:
                            n3 = min(3, len(batch) - j)
                            pt3 = pst.tile([P, 3 * P], F16, name="pt3",
                                           tag="pt3")
                            pts.append((pt3, n3))
                        nc.tensor.transpose(out=pt3[:KC, j3 * P:(j3 + 1) * P],
                                            in_=a16[:], identity=ident[:])
                    aggTs = []
                    for (pt3, n3) in pts:
                        aggT = pw.tile([P, 3 * P], F16, name="aggT",
                                       tag="aggT")
                        nc.scalar.copy(aggT[:KC, :n3 * P], pt3[:KC, :n3 * P])
                        aggTs.append(aggT)
                    for j0 in range(0, len(batch), 2):
                        n2 = min(2, len(batch) - j0)
                        pm2 = pso.tile([P, 4 * F], F32, name="pm2", tag="pm2")
                        for jj in range(n2):
                            j = j0 + jj
                            slot = n2 - 1 - jj     # ascending-t order
                            nc.tensor.matmul(
                                pm2[:, slot * 2 * F:(slot + 1) * 2 * F],
                                lhsT=aggTs[j // 3][:KC,
                                                   (j % 3) * P:(j % 3 + 1) * P],
                                rhs=wab_sb[:], start=True, stop=True)
                        lo_t = batch[j0 + n2 - 1]
                        # PSUM -> fp16 staging on DVE (idle once its share
                        # of the adds is done; scalar engine stays on the
                        # a16/aggT copies)
                        nc.vector.tensor_copy(
                            stage[:, lo_t * 2 * F:(lo_t + n2) * 2 * F],
                            pm2[:, :n2 * 2 * F])

            # interleave groups with the epilogues they release, so no
            # engine queue head-blocks on late data: after group g (layers
            # < B done) tiles with K <= B are final -- a contiguous
            # high-t suffix, emitted immediately and stored as one chunk
            K_arr = list(K)
            released = NTILES        # tiles >= released are already done
            for gi, g in enumerate(groups):
                o0 = int(layer_off[g[0]])
                # fp16 partial sums into the group's first layer block
                for s in g[1:]:
                    off = int(layer_off[s])
                    w = widths[s]
                    wd = min(w, T0f)
                    nc.vector.tensor_add(
                        nbr_sb[:, o0 * F:(o0 + wd) * F],
                        nbr_sb[:, o0 * F:(o0 + wd) * F],
                        nbr_sb[:, off * F:(off + wd) * F],
                    )
                    if w > T0f:
                        nc.gpsimd.tensor_add(
                            nbr_sb[:, (o0 + T0f) * F:(o0 + w) * F],
                            nbr_sb[:, (o0 + T0f) * F:(o0 + w) * F],
                            nbr_sb[:, (off + T0f) * F:(off + w) * F],
                        )
                # group total -> f32 accumulator (group 0 initializes)
                w = widths[g[0]]
                wd = min(w, T0m)
                if gi == 0:
                    # acc init as one exact scalar-engine Copy (fp16->f32),
                    # in Act's idle window: frees ~8.5us of DVE and ~8us of
                    # GPSIMD time and moves the first release earlier
                    nc.scalar.copy(acc[:, :w * F],
                                   nbr_sb[:, o0 * F:(o0 + w) * F])
                else:
                    nc.vector.tensor_add(
                        acc[:, :wd * F], acc[:, :wd * F],
                        nbr_sb[:, o0 * F:(o0 + wd) * F],
                    )
                    if w > T0m:
                        nc.gpsimd.tensor_add(
                            acc[:, T0m * F:w * F], acc[:, T0m * F:w * F],
                            nbr_sb[:, (o0 + T0m) * F:(o0 + w) * F],
                        )
                # release: all layers < B are folded into acc now
                B = g[-1] + 1
                tB = released
                while tB > 0 and K_arr[tB - 1] <= B:
                    tB -= 1
                if gi == len(groups) - 1:
                    tB = 0
                if tB < released:
                    epilogues(list(range(released - 1, tB - 1, -1)))
                    nc.sync.dma_start(
                        outab[:, tB * 2 * F:released * 2 * F],
                        stage[:, tB * 2 * F:released * 2 * F])
                    released = tB

    nc.compile()
    return nc


# --------------------------------------------------------------------------
# kernel entry point
# --------------------------------------------------------------------------

def kernel(x, W1, b1, W2a, b2a, W2b, b2b, edge_index, _profile=False):
    global LAST_EXEC_NS
    x = np.ascontiguousarray(np.asarray(x, dtype=np.float32))
    W1 = np.asarray(W1, dtype=np.float32)
    b1 = np.asarray(b1, dtype=np.float32)
    W2a = np.asarray(W2a, dtype=np.float32)
    b2a = np.asarray(b2a, dtype=np.float32)
    W2b = np.asarray(W2b, dtype=np.float32)
    b2b = np.asarray(b2b, dtype=np.float32)
    edge_index = np.asarray(edge_index)

    pp = _preprocess(edge_index)
    dinv = pp["dinv"]
    orders = pp["orders"]
    NL = pp["NL"]

    key = (NL, tuple(pp["widths"]))
    if key not in _NC_CACHE:
        _NC_CACHE.clear()
        _NC_CACHE[key] = _build(pp["widths"], pp["layer_off"], pp["K"], NL)
    nc = _NC_CACHE[key]

    def pad_wab(w_a, b_a, w_b, b_b):
        wp = np.zeros((KC, 2 * F), dtype=np.float32)
        wp[:F, :F] = w_a
        wp[F, :F] = b_a    # bias channel (paired with const 1.0 in a16 col 96)
        wp[:F, F:] = w_b
        wp[F, F:] = b_b
        return wp.astype(np.float16)

    exec_ns = []

    def build_streams(g32):
        """g32: [N, F] f32 dinv[src]-scaled features -> per-core fp16 stream."""
        g16pad = np.zeros((N + 1, F), dtype=np.float16)
        g16pad[:N] = g32.astype(np.float16)
        nbr_all = g16pad[pp["nidx"]]                   # [8, 128, NL, 96]
        return nbr_all.reshape(NCORES, P, NL * F)

    def launch(nbr_all, w_a, b_a, w_b, b_b):
        wab_p = pad_wab(w_a, b_a, w_b, b_b)
        in_maps = [
            {
                "nbr": nbr_all[c],
                "wab": wab_p,
                "dinvl": pp["dinv_lane"][c],
            }
            for c in range(NCORES)
        ]
        res = run_bass_kernel_spmd(nc, in_maps, core_ids=list(range(NCORES)),
                                   trace=bool(_profile))
        exec_ns.append(res.exec_time_ns)
        return res.results

    def assemble(res, half):
        full = np.zeros((N, F), dtype=np.float32)
        for c in range(NCORES):
            arr = res[c]["outab"].reshape(P, NTILES, 2 * F)
            part = arr[:, :, half * F:(half + 1) * F].transpose(1, 0, 2)
            full[orders[c, :NPC]] = part.reshape(TROWS, F)[:NPC]
        return full

    # ---- launch 1: hidden1 = relu(dinv_dst * (A x) W1 + b1) ----
    # (the linear part runs on device; relu folds into the host-side
    # inter-layer exchange)
    g_x = dinv[:, None] * x
    res1 = launch(build_streams(g_x), W1, b1, W1, b1)
    hidden1 = np.maximum(assemble(res1, 0), 0.0)
    s1 = g_x[pp["hub_srcs"]].sum(axis=0, dtype=np.float32)
    hidden1[HUB] = np.maximum((dinv[HUB] * s1) @ W1 + b1, 0.0)

    # ---- launch 2: mu / logstd from shared aggregation of hidden1 ----
    g_h = dinv[:, None] * hidden1
    res2 = launch(build_streams(g_h), W2a, b2a, W2b, b2b)
    mu = assemble(res2, 0)
    logstd = assemble(res2, 1)
    s2 = g_h[pp["hub_srcs"]].sum(axis=0, dtype=np.float32)
    mu[HUB] = (dinv[HUB] * s2) @ W2a + b2a
    logstd[HUB] = (dinv[HUB] * s2) @ W2b + b2b

    LAST_EXEC_NS = exec_ns
    return mu, logstd
